# revision 13
# baseline (speedup 1.0000x reference)
"""AngularPenaltySMLoss (ArcFace) sharded over 8 TRN2 NeuronCores.

Strategy (classification/tensor parallel, classes sharded 8-way):
  - Host: layout prep — L2-normalize features, scale into fp8e4 range,
    transpose, and cast BOTH x and W to fp8 on the host. Device HBM
    traffic drops 4x vs streaming f32 W (6.4MB/core vs 25.6MB/core) and
    the device-side cast work disappears entirely.
  - Device (per core, SPMD, no collectives):
      * stream its W^T shard [512, 12500] fp8 from HBM in column groups
        on the SP engine's HWDGE queue (keeps Pool free for ALU work),
      * PE: DoubleRow fp8 matmuls into per-consumer PSUM tiles,
      * exp + row-sum split across three engines so none gates the PE.
        Each engine consumes its OWN psum tile — a shared tile makes the
        Tile scheduler's semaphore ticketing serialize the readers and
        stalls the PE ~900ns per row chunk:
          - ScalarE: exact exp via activation, fused accum_out row-sums,
          - Pool: Schraudolph exp on its tile — one tensor_scalar
            (psum * A + B -> int32; the int32 bit pattern IS the f32
            exp approximation) into an SBUF scratch,
          - VectorE: row-sum of the f32-bitcast scratch + final folds.
        The Schraudolph bias constant is tuned for zero mean error on
        the exp-sum; residual row error ~6e-4 rel vs 2e-2 tolerance.
      * output [128, 8] f32 partial exp sums per core.
  - Host: gather partials, exact true-class logit, arcface numerator,
    final scalar loss (all f64).
"""

import sys

if "/opt/trn_rl_repo" not in sys.path:
    sys.path.insert(0, "/opt/trn_rl_repo")

import numpy as np

S = 64.0
MARGIN = 0.5
EPS = 1e-07
B, D, C = 1024, 512, 100000
NCORES = 8
CSH = C // NCORES            # 12500 classes per core (no padding)
CT = 512                     # full c-tile width (one PSUM bank fp32)
NT = 25                      # c-tiles per core (last one ragged: 212)
LAST_CT = CSH - (NT - 1) * CT  # 212
NB = B // 128                # 8 row chunks
KT = D // 128                # 4 contraction chunks (2 DoubleRow passes)
WSCALE = 32.0                # fp8 range scaling for W
XSCALE = 16.0                # fp8 range scaling for normalized x

# Schraudolph exp: exp(z) ~= bitcast_f32(i32(A*z + 127*2^23 - C0)) with
# z = psum/8 (psum = 16x * 32w = 512*logit, exp arg = 64*logit). C0
# tuned numerically for zero mean error of the exp-SUM over the actual
# logit distribution.
SCH_A = float(2.0 ** 23 / np.log(2.0) / 8.0)   # applied to psum
SCH_B = float(127 * 2 ** 23 - 482784.0)

# Column groups streamed from HBM: (start_tile, n_tiles). Small groups
# first so the first matmul starts as early as possible. Within a
# group, the first `act_tiles` tiles land in the ScalarE psum tile and
# the rest in the VectorE psum tile.
GROUPS = [(0, 1, 1), (1, 2, 1), (3, 4, 2), (7, 4, 2), (11, 4, 2),
          (15, 4, 2), (19, 4, 2), (23, 2, 1)]  # (t0, n_tiles, act_tiles)
NGRP = len(GROUPS)
# acc slots: group g -> ACT slot g; DVE slot NGRP+k for the k-th group
# with an offloaded part (g1..g7).
DVE_SLOT = {g: NGRP + k for k, g in enumerate(range(1, NGRP))}
NACC = NGRP + len(DVE_SLOT)


def _tile_width(t):
    return LAST_CT if t == NT - 1 else CT


_CACHE = {}


def _build_nc():
    from contextlib import ExitStack

    import concourse.bacc as bacc
    import concourse.mybir as mybir
    import concourse.tile as tile
    from concourse.tile_rust import add_dep_helper

    f32 = mybir.dt.float32
    f8 = mybir.dt.float8e4
    i32 = mybir.dt.int32
    AF = mybir.ActivationFunctionType
    ALU = mybir.AluOpType

    nc = bacc.Bacc("TRN2", target_bir_lowering=False, debug=False,
                   num_devices=NCORES)

    xt_ext = nc.dram_tensor("xT", [D, B], f8, kind="ExternalInput")
    wt_ext = nc.dram_tensor("wT", [D, CSH], f8, kind="ExternalInput")
    out_ext = nc.dram_tensor("out", [128, NB], f32, kind="ExternalOutput")

    # The Tile scheduler breaks priority ties in hash order, which makes
    # the emitted schedule depend on PYTHONHASHSEED. Pin each engine's
    # stream to program order with order-only deps.
    _prev = {}

    def _chain(key, bi):
        if key in _prev:
            add_dep_helper(bi.ins, _prev[key].ins, sync=False,
                           reason="deterministic program order")
        _prev[key] = bi
        return bi

    with tile.TileContext(nc) as tc, ExitStack() as ctx:
        const_pool = ctx.enter_context(tc.tile_pool(name="const", bufs=1))
        w8_pool = ctx.enter_context(tc.tile_pool(name="w8", bufs=3))
        sch_pool = ctx.enter_context(tc.tile_pool(name="sch", bufs=2))
        psa_pool = ctx.enter_context(
            tc.tile_pool(name="psa", bufs=2, space="PSUM"))
        psb_pool = ctx.enter_context(
            tc.tile_pool(name="psb", bufs=2, space="PSUM"))

        # Force the ACT exp table load at t=0 (it costs ~2.7us; without
        # this it happens on the critical path at the first real exp).
        warm = const_pool.tile([128, 1], f32)
        nc.gpsimd.memset(warm[:], 0.0)
        nc.scalar.activation(warm[:], warm[:], AF.Exp)

        # Features, fp8 straight from HBM (cast on host): xt8[p, k, b] =
        # xn16[b, 128k+p]. Split on the SP queue: the j=0 slice (64KB)
        # goes first so group0's first matmuls unblock ~2us earlier; the
        # rest follows right after W group0.
        xt8 = const_pool.tile([128, KT, B], f8)
        xt_src = xt_ext.ap().rearrange("(k p) b -> p k b", p=128)
        _chain("hdma", nc.sync.dma_start(
            out=xt8[:, :, :128], in_=xt_src[:, :, :128]))

        # Bridge the PE idle window until the first real matmul with
        # throwaway matmuls on a zeroed fp8 tile (no DMA dependency), so
        # the HAM clock gate is warm when group0's matmuls start.
        xwarm = const_pool.tile([128, 2, 128], f8)
        nc.vector.memset(xwarm[:], 0.0)
        warm_ps = psa_pool.tile([128, 2 * CT], f32, tag="psa")
        for r in range(28):
            _chain("pe", nc.tensor.matmul(
                warm_ps[:, :128],
                lhsT=xwarm[:],
                rhs=xwarm[:],
                start=True, stop=True,
                perf_mode=mybir.MatmulPerfMode.DoubleRow,
            ))

        # Per-(row-chunk, slot) partial sums and folded output.
        acc = const_pool.tile([128, NB, NACC], f32)
        out_s = const_pool.tile([128, NB], f32)

        for g, (t0, width, act_tiles) in enumerate(GROUPS):
            widths = [_tile_width(t0 + i) for i in range(width)]
            span = sum(widths)
            acols = sum(widths[:act_tiles])
            pcols = span - acols
            base = t0 * CT

            # One HWDGE job per group: 512 descriptors of `span` bytes.
            w8g = w8_pool.tile([128, KT, 4 * CT], f8, tag="w8g")
            _chain("hdma", nc.sync.dma_start(
                out=w8g[:, :, :span],
                in_=wt_ext.ap()[:, base:base + span]
                .rearrange("(k p) c -> p k c", p=128)))
            if g == 0:
                # Remaining feature rows land while group0 is computing.
                _chain("hdma", nc.sync.dma_start(
                    out=xt8[:, :, 128:], in_=xt_src[:, :, 128:]))

            for j in range(NB):
                psa = psa_pool.tile([128, 2 * CT], f32, tag="psa")
                psb = None
                if pcols:
                    psb = psb_pool.tile([128, 2 * CT], f32, tag="psb",
                                        name="psb")
                for k2 in range(KT // 2):
                    lhsT = xt8[:, 2 * k2:2 * k2 + 2, j * 128:(j + 1) * 128]
                    off = 0
                    for i in range(width):
                        cw = widths[i]
                        if i < act_tiles:
                            dst = psa[:, off:off + cw]
                        else:
                            dst = psb[:, off - acols:off - acols + cw]
                        _chain("pe", nc.tensor.matmul(
                            dst,
                            lhsT=lhsT,
                            rhs=w8g[:, 2 * k2:2 * k2 + 2, off:off + cw],
                            start=(k2 == 0),
                            stop=(k2 == KT // 2 - 1),
                            perf_mode=mybir.MatmulPerfMode.DoubleRow,
                        ))
                        off += cw
                # ScalarE: exact exp in place into PSUM (values never
                # read, only the fused accum row-sums are).
                _chain("act", nc.scalar.activation(
                    psa[:, :acols],
                    psa[:, :acols],
                    AF.Exp,
                    scale=S / (WSCALE * XSCALE),
                    accum_out=acc[:, j, g:g + 1],
                ))
                if pcols:
                    # VectorE: Schraudolph exp affine (GPSIMD cannot
                    # read PSUM on this toolchain). The int32 result's
                    # bit pattern is the f32 exp approximation.
                    sch = sch_pool.tile([128, 2 * CT], i32, tag="sch")
                    _chain("dve", nc.vector.tensor_scalar(
                        out=sch[:, :pcols],
                        in0=psb[:, :pcols],
                        scalar1=SCH_A,
                        scalar2=SCH_B,
                        op0=ALU.mult,
                        op1=ALU.add,
                    ))
                    # VectorE: row-sum of the bitcast exp values.
                    _chain("dve", nc.vector.tensor_reduce(
                        out=acc[:, j, DVE_SLOT[g]:DVE_SLOT[g] + 1],
                        in_=sch[:, :pcols].bitcast(f32),
                        axis=mybir.AxisListType.X,
                        op=ALU.add,
                    ))
                if g == NGRP - 1:
                    # Fold this row-chunk's partial sums while the other
                    # engines still stream the remaining row-chunks.
                    _chain("dve", nc.vector.tensor_reduce(
                        out=out_s[:, j:j + 1], in_=acc[:, j:j + 1, :],
                        axis=mybir.AxisListType.X, op=ALU.add))
                    if j == NB - 2:
                        # Ship the first 7 row chunks early; only j=7
                        # rides the tail.
                        _chain("hdma", nc.sync.dma_start(
                            out=out_ext.ap()[:, :NB - 1],
                            in_=out_s[:, :NB - 1]))

        _chain("hdma", nc.sync.dma_start(
            out=out_ext.ap()[:, NB - 1:], in_=out_s[:, NB - 1:]))

    nc.compile()
    return nc


def _host_inputs(features, W):
    """Host-side layout prep: normalize, scale, transpose, fp8-cast."""
    import ml_dtypes

    f8 = ml_dtypes.float8_e4m3
    x = np.asarray(features, dtype=np.float32)
    Wf = np.asarray(W, dtype=np.float32)

    norms = np.maximum(np.sqrt((x.astype(np.float64) ** 2).sum(1)), 1e-12)
    xn16 = (x.astype(np.float64) * (XSCALE / norms)[:, None]).astype(np.float32)
    xT8 = np.ascontiguousarray(xn16.T).astype(f8)        # [D, B] fp8

    w8 = (Wf * WSCALE).astype(f8)                        # [C, D] fp8
    wT_shards = [
        np.ascontiguousarray(w8[m * CSH:(m + 1) * CSH].T)  # [D, 12500]
        for m in range(NCORES)
    ]
    return xT8, wT_shards, norms


def _finish_host(partials, features, W, y_true, norms):
    """Exact scalar assembly from per-core partial exp sums."""
    x64 = np.asarray(features, dtype=np.float64)
    xn = x64 / norms[:, None]
    Wy = np.asarray(W, dtype=np.float64)[np.asarray(y_true)]
    tgt = np.einsum("bd,bd->b", xn, Wy)

    total = np.zeros(B, dtype=np.float64)
    for p in partials:
        total += p.astype(np.float64).T.reshape(B)

    numerator = S * np.cos(np.arccos(np.clip(tgt, -1.0 + EPS, 1.0 - EPS))
                           + MARGIN)
    excl = total - np.exp(S * tgt)
    denom = np.exp(numerator) + excl
    L = numerator - np.log(denom)
    return np.array(-L.mean(), dtype=np.float32)


def _get_nc():
    if "nc" not in _CACHE:
        _CACHE["nc"] = _build_nc()
    return _CACHE["nc"]


def kernel(features, W, y_true):
    from concourse.bass_utils import run_bass_kernel_spmd

    xT, wT_shards, norms = _host_inputs(features, W)
    in_maps = [{"xT": xT, "wT": wT_shards[m]} for m in range(NCORES)]
    nc = _get_nc()
    res = run_bass_kernel_spmd(nc, in_maps, core_ids=list(range(NCORES)))
    partials = [res.results[m]["out"] for m in range(NCORES)]
    return _finish_host(partials, features, W, y_true, norms)


# revision 14
# speedup vs baseline: 1.0106x; 1.0106x over previous
"""AngularPenaltySMLoss (ArcFace) sharded over 8 TRN2 NeuronCores.

Strategy (classification/tensor parallel, classes sharded 8-way):
  - Host: layout prep — L2-normalize features, scale into fp8e4 range,
    transpose, and cast BOTH x and W to fp8 on the host. Device HBM
    traffic drops 4x vs streaming f32 W (6.4MB/core vs 25.6MB/core) and
    the device-side cast work disappears entirely.
  - Device (per core, SPMD, no collectives):
      * stream its W^T shard [512, 12500] fp8 from HBM in column groups
        on the SP engine's HWDGE queue (keeps Pool free for ALU work),
      * PE: DoubleRow fp8 matmuls into per-consumer PSUM tiles,
      * exp + row-sum split across three engines so none gates the PE.
        Each engine consumes its OWN psum tile — a shared tile makes the
        Tile scheduler's semaphore ticketing serialize the readers and
        stalls the PE ~900ns per row chunk:
          - ScalarE: exact exp via activation, fused accum_out row-sums,
          - Pool: Schraudolph exp on its tile — one tensor_scalar
            (psum * A + B -> int32; the int32 bit pattern IS the f32
            exp approximation) into an SBUF scratch,
          - VectorE: row-sum of the f32-bitcast scratch + final folds.
        The Schraudolph bias constant is tuned for zero mean error on
        the exp-sum; residual row error ~6e-4 rel vs 2e-2 tolerance.
      * output [128, 8] f32 partial exp sums per core.
  - Host: gather partials, exact true-class logit, arcface numerator,
    final scalar loss (all f64).
"""

import sys

if "/opt/trn_rl_repo" not in sys.path:
    sys.path.insert(0, "/opt/trn_rl_repo")

import numpy as np

S = 64.0
MARGIN = 0.5
EPS = 1e-07
B, D, C = 1024, 512, 100000
NCORES = 8
CSH = C // NCORES            # 12500 classes per core (no padding)
CT = 512                     # full c-tile width (one PSUM bank fp32)
NT = 25                      # c-tiles per core (last one ragged: 212)
LAST_CT = CSH - (NT - 1) * CT  # 212
NB = B // 128                # 8 row chunks
KT = D // 128                # 4 contraction chunks (2 DoubleRow passes)
WSCALE = 32.0                # fp8 range scaling for W
XSCALE = 16.0                # fp8 range scaling for normalized x

# Schraudolph exp: exp(z) ~= bitcast_f32(i32(A*z + 127*2^23 - C0)) with
# z = psum/8 (psum = 16x * 32w = 512*logit, exp arg = 64*logit). C0
# tuned numerically for zero mean error of the exp-SUM over the actual
# logit distribution.
SCH_A = float(2.0 ** 23 / np.log(2.0) / 8.0)   # applied to psum
SCH_B = float(127 * 2 ** 23 - 482784.0)

# Column groups streamed from HBM: (start_tile, n_tiles). Small groups
# first so the first matmul starts as early as possible. Within a
# group, the first `act_tiles` tiles land in the ScalarE psum tile and
# the rest in the VectorE psum tile.
GROUPS = [(0, 1, 1), (1, 2, 1), (3, 4, 2), (7, 4, 2), (11, 4, 2),
          (15, 4, 2), (19, 4, 2), (23, 2, 1)]  # (t0, n_tiles, act_tiles)
NGRP = len(GROUPS)
# acc slots: group g -> ACT slot g; DVE slot NGRP+k for the k-th group
# with an offloaded part (g1..g7).
DVE_SLOT = {g: NGRP + k for k, g in enumerate(range(1, NGRP))}
NACC = NGRP + len(DVE_SLOT)


def _tile_width(t):
    return LAST_CT if t == NT - 1 else CT


_CACHE = {}


def _build_nc():
    from contextlib import ExitStack

    import concourse.bacc as bacc
    import concourse.mybir as mybir
    import concourse.tile as tile
    from concourse.tile_rust import add_dep_helper

    f32 = mybir.dt.float32
    f8 = mybir.dt.float8e4
    i32 = mybir.dt.int32
    AF = mybir.ActivationFunctionType
    ALU = mybir.AluOpType

    nc = bacc.Bacc("TRN2", target_bir_lowering=False, debug=False,
                   num_devices=NCORES)

    xt_ext = nc.dram_tensor("xT", [D, B], f8, kind="ExternalInput")
    wt_ext = nc.dram_tensor("wT", [D, CSH], f8, kind="ExternalInput")
    out_ext = nc.dram_tensor("out", [128, NB], f32, kind="ExternalOutput")

    # The Tile scheduler breaks priority ties in hash order, which makes
    # the emitted schedule depend on PYTHONHASHSEED. Pin each engine's
    # stream to program order with order-only deps.
    _prev = {}

    def _chain(key, bi):
        if key in _prev:
            add_dep_helper(bi.ins, _prev[key].ins, sync=False,
                           reason="deterministic program order")
        _prev[key] = bi
        return bi

    with tile.TileContext(nc) as tc, ExitStack() as ctx:
        const_pool = ctx.enter_context(tc.tile_pool(name="const", bufs=1))
        w8_pool = ctx.enter_context(tc.tile_pool(name="w8", bufs=3))
        sch_pool = ctx.enter_context(tc.tile_pool(name="sch", bufs=2))
        psa_pool = ctx.enter_context(
            tc.tile_pool(name="psa", bufs=2, space="PSUM"))
        psb_pool = ctx.enter_context(
            tc.tile_pool(name="psb", bufs=2, space="PSUM"))

        # Force the ACT exp table load at t=0 (it costs ~2.7us; without
        # this it happens on the critical path at the first real exp).
        warm = const_pool.tile([128, 1], f32)
        nc.gpsimd.memset(warm[:], 0.0)
        nc.scalar.activation(warm[:], warm[:], AF.Exp)

        # Features, fp8 straight from HBM (cast on host): xt8[p, k, b] =
        # xn16[b, 128k+p]. Split on the SP queue: the j=0 slice (64KB)
        # goes first so group0's first matmuls unblock ~2us earlier; the
        # rest follows right after W group0.
        xt8 = const_pool.tile([128, KT, B], f8)
        xt_src = xt_ext.ap().rearrange("(k p) b -> p k b", p=128)
        _chain("hdma", nc.sync.dma_start(
            out=xt8[:, :, :128], in_=xt_src[:, :, :128]))

        # Bridge the PE idle window until the first real matmul with
        # throwaway matmuls on a zeroed fp8 tile (no DMA dependency), so
        # the HAM clock gate is warm when group0's matmuls start.
        xwarm = const_pool.tile([128, 2, 128], f8)
        nc.vector.memset(xwarm[:], 0.0)
        warm_ps = psa_pool.tile([128, 2 * CT], f32, tag="psa")
        for r in range(28):
            _chain("pe", nc.tensor.matmul(
                warm_ps[:, :128],
                lhsT=xwarm[:],
                rhs=xwarm[:],
                start=True, stop=True,
                perf_mode=mybir.MatmulPerfMode.DoubleRow,
            ))

        # Per-(row-chunk, slot) partial sums and folded output.
        acc = const_pool.tile([128, NB, NACC], f32)
        out_s = const_pool.tile([128, NB], f32)

        for g, (t0, width, act_tiles) in enumerate(GROUPS):
            widths = [_tile_width(t0 + i) for i in range(width)]
            span = sum(widths)
            acols = sum(widths[:act_tiles])
            pcols = span - acols
            base = t0 * CT

            # One HWDGE job per group: 512 descriptors of `span` bytes.
            w8g = w8_pool.tile([128, KT, 4 * CT], f8, tag="w8g")
            _chain("hdma", nc.sync.dma_start(
                out=w8g[:, :, :span],
                in_=wt_ext.ap()[:, base:base + span]
                .rearrange("(k p) c -> p k c", p=128)))
            if g == 0:
                # Remaining feature rows land while group0 is computing.
                _chain("hdma", nc.sync.dma_start(
                    out=xt8[:, :, 128:], in_=xt_src[:, :, 128:]))

            for j in range(NB):
                psa = psa_pool.tile([128, 2 * CT], f32, tag="psa")
                psb = None
                if pcols:
                    psb = psb_pool.tile([128, 2 * CT], f32, tag="psb",
                                        name="psb")
                # psb tiles go FIRST in each burst: the next row chunk's
                # first matmuls then wait on the fast DVE consumer, and
                # the slower ScalarE consumer gets two extra matmul slots
                # before its psum tile is needed again.
                order = [i for i in range(width) if i >= act_tiles] + \
                        list(range(act_tiles))
                offs = np.cumsum([0] + widths).tolist()
                for k2 in range(KT // 2):
                    lhsT = xt8[:, 2 * k2:2 * k2 + 2, j * 128:(j + 1) * 128]
                    for i in order:
                        cw = widths[i]
                        off = offs[i]
                        if i < act_tiles:
                            dst = psa[:, off:off + cw]
                        else:
                            dst = psb[:, off - acols:off - acols + cw]
                        _chain("pe", nc.tensor.matmul(
                            dst,
                            lhsT=lhsT,
                            rhs=w8g[:, 2 * k2:2 * k2 + 2, off:off + cw],
                            start=(k2 == 0),
                            stop=(k2 == KT // 2 - 1),
                            perf_mode=mybir.MatmulPerfMode.DoubleRow,
                        ))
                        off += cw
                # ScalarE: exact exp in place into PSUM (values never
                # read, only the fused accum row-sums are).
                _chain("act", nc.scalar.activation(
                    psa[:, :acols],
                    psa[:, :acols],
                    AF.Exp,
                    scale=S / (WSCALE * XSCALE),
                    accum_out=acc[:, j, g:g + 1],
                ))
                if pcols:
                    # VectorE: Schraudolph exp affine (GPSIMD cannot
                    # read PSUM on this toolchain). The int32 result's
                    # bit pattern is the f32 exp approximation.
                    sch = sch_pool.tile([128, 2 * CT], i32, tag="sch")
                    _chain("dve", nc.vector.tensor_scalar(
                        out=sch[:, :pcols],
                        in0=psb[:, :pcols],
                        scalar1=SCH_A,
                        scalar2=SCH_B,
                        op0=ALU.mult,
                        op1=ALU.add,
                    ))
                    # VectorE: row-sum of the bitcast exp values.
                    _chain("dve", nc.vector.tensor_reduce(
                        out=acc[:, j, DVE_SLOT[g]:DVE_SLOT[g] + 1],
                        in_=sch[:, :pcols].bitcast(f32),
                        axis=mybir.AxisListType.X,
                        op=ALU.add,
                    ))
                if g == NGRP - 1:
                    # Fold this row-chunk's partial sums while the other
                    # engines still stream the remaining row-chunks.
                    _chain("dve", nc.vector.tensor_reduce(
                        out=out_s[:, j:j + 1], in_=acc[:, j:j + 1, :],
                        axis=mybir.AxisListType.X, op=ALU.add))
                    if j == NB - 2:
                        # Ship the first 7 row chunks early; only j=7
                        # rides the tail.
                        _chain("hdma", nc.sync.dma_start(
                            out=out_ext.ap()[:, :NB - 1],
                            in_=out_s[:, :NB - 1]))

        _chain("hdma", nc.sync.dma_start(
            out=out_ext.ap()[:, NB - 1:], in_=out_s[:, NB - 1:]))

    nc.compile()
    return nc


def _host_inputs(features, W):
    """Host-side layout prep: normalize, scale, transpose, fp8-cast."""
    import ml_dtypes

    f8 = ml_dtypes.float8_e4m3
    x = np.asarray(features, dtype=np.float32)
    Wf = np.asarray(W, dtype=np.float32)

    norms = np.maximum(np.sqrt((x.astype(np.float64) ** 2).sum(1)), 1e-12)
    xn16 = (x.astype(np.float64) * (XSCALE / norms)[:, None]).astype(np.float32)
    xT8 = np.ascontiguousarray(xn16.T).astype(f8)        # [D, B] fp8

    w8 = (Wf * WSCALE).astype(f8)                        # [C, D] fp8
    wT_shards = [
        np.ascontiguousarray(w8[m * CSH:(m + 1) * CSH].T)  # [D, 12500]
        for m in range(NCORES)
    ]
    return xT8, wT_shards, norms


def _finish_host(partials, features, W, y_true, norms):
    """Exact scalar assembly from per-core partial exp sums."""
    x64 = np.asarray(features, dtype=np.float64)
    xn = x64 / norms[:, None]
    Wy = np.asarray(W, dtype=np.float64)[np.asarray(y_true)]
    tgt = np.einsum("bd,bd->b", xn, Wy)

    total = np.zeros(B, dtype=np.float64)
    for p in partials:
        total += p.astype(np.float64).T.reshape(B)

    numerator = S * np.cos(np.arccos(np.clip(tgt, -1.0 + EPS, 1.0 - EPS))
                           + MARGIN)
    excl = total - np.exp(S * tgt)
    denom = np.exp(numerator) + excl
    L = numerator - np.log(denom)
    return np.array(-L.mean(), dtype=np.float32)


def _get_nc():
    if "nc" not in _CACHE:
        _CACHE["nc"] = _build_nc()
    return _CACHE["nc"]


def kernel(features, W, y_true):
    from concourse.bass_utils import run_bass_kernel_spmd

    xT, wT_shards, norms = _host_inputs(features, W)
    in_maps = [{"xT": xT, "wT": wT_shards[m]} for m in range(NCORES)]
    nc = _get_nc()
    res = run_bass_kernel_spmd(nc, in_maps, core_ids=list(range(NCORES)))
    partials = [res.results[m]["out"] for m in range(NCORES)]
    return _finish_host(partials, features, W, y_true, norms)
